# revision 13
# baseline (speedup 1.0000x reference)
"""CrossAttention Trainium2 kernel.

Full inputs -> shard (batch x head-group) over 8 NeuronCores -> Bass/Tile
kernel per core -> host-side partial-sum + bias.

Per-core shard (core c): batch b = c // 4, heads hg = (c % 4) * 4 .. +4.
Each core computes, for its 4 heads:
  QT = (x_b @ Wq[:, cols])^T          [256, 2048]  (f32r)
  KT = (ctx_b @ Wk[:, cols])^T        [256, 2048]  (f32r)
  V' = [ctx_b @ Wv[:, cols] | 1]      [2048, 4, 65] (fp16, ones col -> denom)
  per head: S^T = K_h Q_h^T (f32r), P = exp(S^T * scale) (fp16),
            O'^T = V'^T P^T  (PSUM fp32; row 64 = softmax denominator)
  normalize O by 1/denom (broadcast via tiny rank-2 matmul), then
  partial_out = O_norm^T.T @ Wout[rows] -> [2048, 1024]  (f32r matmuls)
Host: out[b] = sum of the 4 partials + bias.
"""

import numpy as np

import concourse.bass as bass
import concourse.bacc as bacc
import concourse.mybir as mybir
import concourse.tile as tile
from concourse import bass_utils

B, S, D = 2, 2048, 1024
H_TOT, DH = 16, 64
N_CORES = 8
H = 4                 # heads per core
I = H * DH            # 256: inner dim per core
SCALE = float(DH) ** -0.5
KD = D // 128         # 8 contraction tiles over model dim
NQ = S // 512         # 4 query chunks of 512
NKV = S // 128        # 16 kv tiles of 128

F32 = mybir.dt.float32
F32R = mybir.dt.float32r
F16 = mybir.dt.float16

AF = mybir.ActivationFunctionType

_CACHE = {}


def _emit(nc, tc, xT, cT, wq, wk, wv, wo, out):
    pp = tc.alloc_tile_pool(name="persist", bufs=1)

    # ---- persistent SBUF ----
    qt = [pp.tile([128, S], F32R, tag=f"qt{m}", name=f"qt{m}") for m in range(2)]
    kt = [pp.tile([128, S], F32R, tag=f"kt{m}", name=f"kt{m}") for m in range(2)]
    vp = pp.tile([128, NKV, H, 65], F16, tag="vp")
    # per-head softmax denominators, free-dim concat on one partition.
    # fp16: D ~ 3e3 << 65504, and 5e-4 rounding on the denominator is fine.
    d_cat = pp.tile([1, H * S], F16, tag="d_cat")
    # unnormalized O^T halves; stored f32r (DVE eviction rounds) so the
    # in-place normalize and the out-projection can consume them directly
    o_un = [pp.tile([128, S], F32R, tag=f"o_un{m}", name=f"o_un{m}") for m in range(2)]

    # ones column of V' (softmax denominator trick)
    nc.vector.memset(vp[:, :, :, 64], 1.0)

    # ================= Phase 1: projections =================
    with (
        tc.tile_pool(name="ph1", bufs=1) as p1,
        tc.tile_pool(name="ps1", bufs=1, space="PSUM") as ps1,
    ):
        wq_s = p1.tile([128, KD, I], F32R, tag="wq")
        wk_s = p1.tile([128, KD, I], F32R, tag="wk")
        wv_s = p1.tile([128, KD, I], F32R, tag="wv")
        nc.sync.dma_start(wq_s[:], wq.rearrange("(k p) i -> p k i", p=128))
        nc.sync.dma_start(wk_s[:], wk.rearrange("(k p) i -> p k i", p=128))
        nc.sync.dma_start(wv_s[:], wv.rearrange("(k p) i -> p k i", p=128))

        ct_s = p1.tile([128, KD, S], F32R, tag="ct")
        for k in range(KD):
            nc.sync.dma_start(ct_s[:, k], cT[k * 128:(k + 1) * 128, :])

        # -- 1a: QT = Wq^T x^T, xT streamed K-outer into 8 psum accumulators
        with tc.tile_pool(name="xs", bufs=3) as xsp:
            ps_q = [ps1.tile([128, 512], F32, tag=f"pj{j}", name=f"psq{j}")
                    for j in range(8)]
            for k in range(KD):
                xt_k = xsp.tile([128, S], F32R, tag="xt", name="xt_k")
                nc.sync.dma_start(xt_k[:], xT[k * 128:(k + 1) * 128, :])
                for m in range(2):
                    for qc in range(NQ):
                        nc.tensor.matmul(
                            ps_q[m * NQ + qc][:],
                            wq_s[:, k, m * 128:(m + 1) * 128],
                            xt_k[:, qc * 512:(qc + 1) * 512],
                            start=(k == 0), stop=(k == KD - 1),
                        )
            for m in range(2):
                for qc in range(NQ):
                    nc.vector.tensor_copy(
                        qt[m][:, qc * 512:(qc + 1) * 512], ps_q[m * NQ + qc][:])

        # -- 1b: KT, same pattern with resident ctx^T
        ps_k = [ps1.tile([128, 512], F32, tag=f"pj{j}", name=f"psk{j}")
                for j in range(8)]
        for k in range(KD):
            for m in range(2):
                for qc in range(NQ):
                    nc.tensor.matmul(
                        ps_k[m * NQ + qc][:],
                        wk_s[:, k, m * 128:(m + 1) * 128],
                        ct_s[:, k, qc * 512:(qc + 1) * 512],
                        start=(k == 0), stop=(k == KD - 1),
                    )
        for m in range(2):
            for qc in range(NQ):
                nc.vector.tensor_copy(
                    kt[m][:, qc * 512:(qc + 1) * 512], ps_k[m * NQ + qc][:])

        # -- 1c: V natural [kv, 256] -> fp16 V' with per-head stride 65
        for mv in range(NKV):
            ps_v = ps1.tile([128, 256], F32, tag=f"pj{mv % 4}", name="psv")
            for k in range(KD):
                nc.tensor.matmul(
                    ps_v[:],
                    ct_s[:, k, mv * 128:(mv + 1) * 128],
                    wv_s[:, k, :],
                    start=(k == 0), stop=(k == KD - 1),
                )
            nc.vector.tensor_copy(
                vp[:, mv, :, 0:64],
                ps_v.rearrange("p (h d) -> p h d", h=H),
            )

    # ================= Phase 2: attention per head =================
    with (
        tc.tile_pool(name="ph2", bufs=1) as p2,
        tc.tile_pool(name="ps_sc", bufs=2, space="PSUM") as ps_sc,
        tc.tile_pool(name="ps_o", bufs=1, space="PSUM") as ps_o,
    ):
        wo_s = p2.tile([128, 2, D], F32R, tag="wo")
        nc.sync.dma_start(wo_s[:], wo.rearrange("(k p) d -> p k d", p=128))
        for h in range(H):
            mt, po = h // 2, (h % 2) * 64
            P = p2.tile([128, NKV, S], F16, tag="P", name=f"P{h}")
            # scores + exp, per kv tile, q groups of 1024
            for i in range(NKV):
                for g in range(2):
                    sg = ps_sc.tile([128, 1024], F32, tag="sc", name="sg")
                    for sub in range(2):
                        q0 = g * 1024 + sub * 512
                        nc.tensor.matmul(
                            sg[:, sub * 512:(sub + 1) * 512],
                            kt[mt][po:po + 64, i * 128:(i + 1) * 128],
                            qt[mt][po:po + 64, q0:q0 + 512],
                            start=True, stop=True,
                        )
                    nc.scalar.activation(
                        P[:, i, g * 1024:(g + 1) * 1024], sg[:],
                        AF.Exp, scale=SCALE,
                    )
            # O'^T = V'^T P^T accumulated over kv tiles; row 64 = denom
            o_ps = ps_o.tile([65, S], F32, tag="o", name=f"ops{h}")
            for qc in range(NQ):
                for i in range(NKV):
                    nc.tensor.matmul(
                        o_ps[:, qc * 512:(qc + 1) * 512],
                        vp[:, i, h, :],
                        P[:, i, qc * 512:(qc + 1) * 512],
                        start=(i == 0), stop=(i == NKV - 1),
                    )
            # evict numerator rows + denominator row (partition-offset copies)
            nc.vector.tensor_copy(o_un[mt][po:po + 64, :], o_ps[0:64, :])
            nc.vector.tensor_copy(d_cat[0:1, h * S:(h + 1) * S], o_ps[64:65, :])

        # ================= Phase 3: normalize + out-projection =========
        # r = 1/D = exp(-ln(D)); rank-1 matmul (ones ⊗ r) broadcasts r over
        # all 128 partitions, then each head's 64-row half is scaled.
        ones = pp.tile([1, 128], F16, tag="ones")
        nc.vector.memset(ones[:], 1.0)

        nc.scalar.activation(d_cat[:], d_cat[:], AF.Ln)  # in-place ln(D)
        r_cat = p2.tile([1, H * S], F16, tag="r_cat", name="r_cat")
        nc.scalar.activation(r_cat[:], d_cat[:], AF.Exp, scale=-1.0)

        for h in range(H):
            mt, po = h // 2, (h % 2) * 64
            for g in range(2):
                R_ps = ps_sc.tile([128, 1024], F32, tag="sc", name="R_ps")
                for sub in range(2):
                    q0 = g * 1024 + sub * 512
                    nc.tensor.matmul(
                        R_ps[:, sub * 512:(sub + 1) * 512],
                        ones[:], r_cat[0:1, h * S + q0:h * S + q0 + 512],
                        start=True, stop=True,
                    )
                # in-place normalize this head's 64 rows
                nc.vector.tensor_mul(
                    o_un[mt][po:po + 64, g * 1024:(g + 1) * 1024],
                    o_un[mt][po:po + 64, g * 1024:(g + 1) * 1024],
                    R_ps[po:po + 64, :],
                )

        # partial_out[q, d] += o_norm[:, q].T @ wo
        with tc.tile_pool(name="ostage", bufs=4) as osp:
            for qi in range(S // 128):
                for n in range(2):
                    op = ps_sc.tile([128, 512], F32, tag="sc", name="op")
                    for m in range(2):
                        nc.tensor.matmul(
                            op[:],
                            o_un[m][:, qi * 128:(qi + 1) * 128],
                            wo_s[:, m, n * 512:(n + 1) * 512],
                            start=(m == 0), stop=(m == 1),
                        )
                    ost = osp.tile([128, 512], F32, tag="ost", name="ost")
                    nc.scalar.copy(ost[:], op[:])
                    nc.sync.dma_start(
                        out[qi * 128:(qi + 1) * 128, n * 512:(n + 1) * 512],
                        ost[:])

    pp.release()


def build_program():
    nc = bacc.Bacc("TRN2", target_bir_lowering=False, debug=False,
                   num_devices=N_CORES)
    xT = nc.dram_tensor("xT", [D, S], F32R, kind="ExternalInput").ap()
    cT = nc.dram_tensor("cT", [D, S], F32R, kind="ExternalInput").ap()
    wq = nc.dram_tensor("wq", [D, I], F32R, kind="ExternalInput").ap()
    wk = nc.dram_tensor("wk", [D, I], F32R, kind="ExternalInput").ap()
    wv = nc.dram_tensor("wv", [D, I], F32R, kind="ExternalInput").ap()
    wo = nc.dram_tensor("wo", [I, D], F32R, kind="ExternalInput").ap()
    out = nc.dram_tensor("out", [S, D], F32, kind="ExternalOutput").ap()
    with tile.TileContext(nc) as tc:
        _emit(nc, tc, xT, cT, wq, wk, wv, wo, out)
    nc.compile()
    return nc


def make_in_maps(x, context, Wq, Wkv, Wout):
    x = np.asarray(x, dtype=np.float32)
    context = np.asarray(context, dtype=np.float32)
    Wq = np.asarray(Wq, dtype=np.float32)
    Wkv = np.asarray(Wkv, dtype=np.float32)
    Wout = np.asarray(Wout, dtype=np.float32)
    in_maps = []
    for c in range(N_CORES):
        b, hg = c // 4, (c % 4) * H
        cols = slice(hg * DH, (hg + H) * DH)
        in_maps.append({
            "xT": np.ascontiguousarray(x[b].T),
            "cT": np.ascontiguousarray(context[b].T),
            "wq": np.ascontiguousarray(Wq[:, cols]),
            "wk": np.ascontiguousarray(Wkv[:, cols]),
            "wv": np.ascontiguousarray(Wkv[:, D + cols.start:D + cols.stop]),
            "wo": np.ascontiguousarray(Wout[cols, :]),
        })
    return in_maps


def kernel(x, context, Wq, Wkv, Wout, bout):
    if "nc" not in _CACHE:
        _CACHE["nc"] = build_program()
    nc = _CACHE["nc"]
    in_maps = make_in_maps(x, context, Wq, Wkv, Wout)
    res = bass_utils.run_bass_kernel_spmd(nc, in_maps, core_ids=list(range(N_CORES)))
    bout = np.asarray(bout, dtype=np.float32)
    out = np.empty((B, S, D), dtype=np.float32)
    for b in range(B):
        acc = res.results[4 * b]["out"].astype(np.float32)
        for c in range(4 * b + 1, 4 * b + 4):
            acc = acc + res.results[c]["out"]
        out[b] = acc + bout[None, :]
    return out
